# revision 7
# baseline (speedup 1.0000x reference)
"""MultiHeadAttention Trainium2 Bass kernel.

Problem: B=8, H=W=32 (S=1024), C=512, 8 heads x 64 dim.
Sharding: data-parallel over batch, one batch element per NeuronCore (8 cores).

Per-core pipeline (batch b):
  Phase A (projections): for x in {q,k,v}: DMA x [1024,512] -> PE-transpose
    to xT [c,s] (fp32, 2cyc/row) -> float32r matmuls with W stationary:
      QT[d,s], KT[d,s] (head dims on partitions), V[s,d] natural (+ones col).
    Biases folded in as K=1 rank-1 matmul accumulated into the same PSUM.
  Phase B (attention, per head pair / q-half): scoresT[k,q] = KT_h^T-stationary
    matmul (K=64, head pairs packed in PE row groups); exp via ACT straight
    from PSUM with the 1/8 scale folded in (no max-subtraction: scores ~N(0,1));
    att@V as V_aug-stationary matmul accumulating over k chunks, the ones
    column producing the softmax denominator for free; PE back-transpose of
    [65, q] tiles; DVE reciprocal + per-partition scale into staged output.
  Phase C: one batched 2MB output DMA.

float32r: single-pass fp32 matmul mode (1 col/cycle at N>=256, vs 4 for
fp32); operands must be written by rounding producers (DVE copies with f32r
output dtype). Measured accuracy ~7e-4 absmax on unit-scale data.
"""
import sys

import numpy as np

if "/opt/trn_rl_repo" not in sys.path:
    sys.path.insert(0, "/opt/trn_rl_repo")

import concourse.bacc as bacc
import concourse.mybir as mybir
import concourse.tile as tile
from concourse import masks
from concourse.bass_utils import run_bass_kernel_spmd

B, HS, WS, C = 8, 32, 32, 512
S = HS * WS          # 1024
D = 512
HEADS = 8
HD = 64              # head dim
N_CORES = 8

f32 = mybir.dt.float32
f32r = mybir.dt.float32r
Exp = mybir.ActivationFunctionType.Exp


def build_nc():
    nc = bacc.Bacc("TRN2", target_bir_lowering=False, debug=False,
                   num_devices=N_CORES)

    x_d = {}
    w_d = {}
    b_d = {}
    for name in ("q", "k", "v"):
        x_d[name] = nc.dram_tensor(f"{name}_in", [S, C], f32, kind="ExternalInput")
        w_d[name] = nc.dram_tensor(f"W{name}", [C, D], f32, kind="ExternalInput")
        b_d[name] = nc.dram_tensor(f"b{name}", [D], f32, kind="ExternalInput")
    out_d = nc.dram_tensor("out", [S, D], f32, kind="ExternalOutput")

    with tile.TileContext(nc) as tc:
        with (
            tc.tile_pool(name="const", bufs=1) as cpool,
            tc.tile_pool(name="xin", bufs=2) as xin_pool,
            tc.tile_pool(name="wbuf", bufs=2) as w_pool,
            tc.tile_pool(name="proj", bufs=1) as proj_pool,
            tc.tile_pool(name="att", bufs=1) as att_pool,
            tc.tile_pool(name="ot", bufs=3) as ot_pool,
            tc.tile_pool(name="ostage", bufs=1) as o_pool,
        ):
            ident = cpool.tile([128, 128], f32)
            masks.make_identity(nc, ident[:])
            ones_sb = cpool.tile([128, 512], f32)
            nc.vector.memset(ones_sb[:], 1.0)
            ones_r = cpool.tile([1, 512], f32r)
            nc.vector.tensor_copy(ones_r[:], ones_sb[0:1, :])

            # Persistent projection outputs
            QT = proj_pool.tile([128, 4, S], f32r, name="QT")   # [d%128, d//128, s]
            KT = proj_pool.tile([128, 4, S], f32r, name="KT")
            # V_aug: [s%128, s//128, head, 65]; col 64 = 1.0 (denominator)
            V = proj_pool.tile([128, 8, HEADS, HD + 1], f32r, name="V")
            nc.vector.tensor_copy(
                V[:, :, :, HD:HD + 1],
                ones_sb[:, 0:64].rearrange("p (a b o) -> p a b o", a=8, b=8))
            o_stage = o_pool.tile([128, 8, D], f32, name="o_stage")

            # ---------------- Phase A: projections ----------------
            with (
                tc.tile_pool(name="ps_tr", bufs=2, space="PSUM") as ps_tr,
                tc.tile_pool(name="ps_proj", bufs=3, space="PSUM") as ps_proj,
                tc.tile_pool(name="xT", bufs=2) as xt_pool,
            ):
                for name, tgt in (("q", QT), ("k", KT), ("v", V)):
                    x_r = x_d[name][:].rearrange("(t p) c -> p t c", p=128)
                    w_sb = w_pool.tile([128, 4, D], f32, name=f"w_{name}",
                                       tag="w_sb")
                    nc.sync.dma_start(
                        w_sb[:], w_d[name][:].rearrange("(cc p) d -> p cc d", p=128))
                    w_r = w_pool.tile([128, 4, D], f32r, name=f"wr_{name}",
                                      tag="w_r")
                    nc.vector.tensor_copy(w_r[:], w_sb[:])
                    b_sb = w_pool.tile([1, D], f32, name=f"b_{name}", tag="b_sb")
                    nc.sync.dma_start(b_sb[:], b_d[name][:].rearrange("(o d) -> o d", o=1))
                    b_r = w_pool.tile([1, D], f32r, name=f"br_{name}", tag="b_r")
                    nc.vector.tensor_copy(b_r[:], b_sb[:])

                    # transpose x -> xT [c%128, cc, s] (f32r via DVE rounding copy)
                    xT = xt_pool.tile([128, 4, S], f32r, name=f"xT_{name}",
                                      tag="xT")
                    for sh in range(2):
                        x_sb = xin_pool.tile([128, 4, C], f32,
                                             name=f"x_{name}{sh}", tag="x_sb")
                        nc.sync.dma_start(x_sb[:], x_r[:, sh * 4:(sh + 1) * 4, :])
                        for ti in range(4):
                            t = sh * 4 + ti
                            pst = ps_tr.tile([128, 4, 128], f32, tag="pst")
                            for cc in range(4):
                                nc.tensor.transpose(
                                    pst[:, cc, :],
                                    x_sb[:, ti, cc * 128:(cc + 1) * 128], ident[:])
                            nc.vector.tensor_copy(
                                xT[:, :, t * 128:(t + 1) * 128], pst[:])

                    if name in ("q", "k"):
                        # tgt[d%128, dt, s] = sum_cc W[cc,dt].T @ xT[cc, s] + b
                        for dt in range(4):
                            for qh in range(2):
                                psq = ps_proj.tile([128, 512], f32, tag="psq")
                                for cc in range(4):
                                    nc.tensor.matmul(
                                        psq[:],
                                        w_r[:, cc, dt * 128:(dt + 1) * 128],
                                        xT[:, cc, qh * 512:(qh + 1) * 512],
                                        start=(cc == 0), stop=False)
                                nc.tensor.matmul(
                                    psq[:],
                                    b_r[0:1, dt * 128:(dt + 1) * 128],
                                    ones_r[0:1, :],
                                    start=False, stop=True)
                                nc.vector.tensor_copy(
                                    tgt[:, dt, qh * 512:(qh + 1) * 512], psq[:])
                    else:
                        # V natural: [s%128, st, d] = xT[cc, st].T-stationary
                        for st in range(8):
                            psv = ps_proj.tile([128, 512], f32, tag="psq")
                            for cc in range(4):
                                nc.tensor.matmul(
                                    psv[:],
                                    xT[:, cc, st * 128:(st + 1) * 128],
                                    w_r[:, cc, :],
                                    start=(cc == 0), stop=False)
                            nc.tensor.matmul(
                                psv[:], ones_r[0:1, 0:128], b_r[0:1, :],
                                start=False, stop=True)
                            nc.vector.tensor_copy(
                                tgt[:, st, :, 0:HD],
                                psv[:].rearrange("p (h e) -> p h e", h=HEADS))

            # ---------------- Phase B: attention ----------------
            with (
                tc.tile_pool(name="ps_s", bufs=1, space="PSUM") as ps_s,
                tc.tile_pool(name="ps_o", bufs=1, space="PSUM") as ps_o,
                tc.tile_pool(name="ps_bt", bufs=2, space="PSUM") as ps_bt,
            ):
                for hp in range(4):          # head pairs share a d-chunk
                    heads = (2 * hp, 2 * hp + 1)
                    for qh in range(2):      # q halves of 512
                        attT = {}
                        for i, h in enumerate(heads):
                            attT[h] = att_pool.tile(
                                [128, 8, 512], f32r, name=f"attT{h}_{qh}",
                                tag=f"attT{i}")
                        # scoresT: per kt pair, two heads packed in row groups
                        for ktp in range(4):
                            pss = {}
                            for i, h in enumerate(heads):
                                pss[h] = ps_s.tile([128, 2, 512], f32,
                                                   name=f"pss{h}_{qh}_{ktp}",
                                                   tag=f"pss{i}")
                            for kt2 in range(2):
                                kt = ktp * 2 + kt2
                                for h in heads:
                                    po = (h % 2) * HD
                                    nc.tensor.matmul(
                                        pss[h][:, kt2, :],
                                        KT[po:po + HD, hp, kt * 128:(kt + 1) * 128],
                                        QT[po:po + HD, hp, qh * 512:(qh + 1) * 512],
                                        start=True, stop=True)
                            for h in heads:
                                nc.scalar.activation(
                                    attT[h][:, ktp * 2:ktp * 2 + 2, :],
                                    pss[h][:], Exp, scale=0.125)
                        # att @ V_aug (+denominator row)
                        pso = {}
                        for i, h in enumerate(heads):
                            pso[h] = ps_o.tile([HD + 1, 512], f32,
                                               name=f"pso{h}_{qh}", tag=f"pso{i}")
                        for kc in range(8):
                            for h in heads:
                                nc.tensor.matmul(
                                    pso[h][:],
                                    V[:, kc, h, :],
                                    attT[h][:, kc, :],
                                    start=(kc == 0), stop=(kc == 7))
                        for h in heads:
                            oT = ot_pool.tile([HD + 1, 512], f32, tag="oT")
                            nc.vector.tensor_copy(oT[:], pso[h][:])
                            # back-transpose 4 q-subtiles into one PSUM bank
                            pbt = ps_bt.tile([128, 4, HD + 1], f32, tag="pbt")
                            for qs in range(4):
                                nc.tensor.transpose(
                                    pbt[:, qs, :],
                                    oT[:, qs * 128:(qs + 1) * 128],
                                    ident[0:HD + 1, 0:HD + 1])
                            rec = ot_pool.tile([128, 4], f32, tag="rec")
                            nc.vector.reciprocal(rec[:], pbt[:, :, HD])
                            for qs in range(4):
                                qt = qh * 4 + qs
                                nc.vector.tensor_scalar_mul(
                                    o_stage[:, qt, h * HD:(h + 1) * HD],
                                    pbt[:, qs, 0:HD],
                                    rec[:, qs:qs + 1])

            # ---------------- Phase C: output ----------------
            nc.sync.dma_start(
                out_d[:].rearrange("(t p) d -> p t d", p=128), o_stage[:])

    nc.compile()
    return nc


_NC = None


def _get_nc():
    global _NC
    if _NC is None:
        _NC = build_nc()
    return _NC


def _make_in_maps(inputs):
    in_maps = []
    for b in range(B):
        m = {
            "q_in": np.ascontiguousarray(inputs["q_in"][b].reshape(S, C)),
            "k_in": np.ascontiguousarray(inputs["k_in"][b].reshape(S, C)),
            "v_in": np.ascontiguousarray(inputs["v_in"][b].reshape(S, C)),
            "Wq": np.asarray(inputs["Wq"]), "bq": np.asarray(inputs["bq"]),
            "Wk": np.asarray(inputs["Wk"]), "bk": np.asarray(inputs["bk"]),
            "Wv": np.asarray(inputs["Wv"]), "bv": np.asarray(inputs["bv"]),
        }
        in_maps.append(m)
    return in_maps


def kernel(**inputs):
    nc = _get_nc()
    res = run_bass_kernel_spmd(nc, _make_in_maps(inputs), list(range(N_CORES)))
    out = np.stack([res.results[i]["out"] for i in range(B)])
    return out.reshape(B, HS, WS, D).astype(np.float32)


if __name__ == "__main__":
    rng = np.random.default_rng(0)
    ins = {
        "q_in": rng.standard_normal((B, HS, WS, C), dtype=np.float32),
        "k_in": rng.standard_normal((B, HS, WS, C), dtype=np.float32),
        "v_in": rng.standard_normal((B, HS, WS, C), dtype=np.float32),
        "Wq": (rng.standard_normal((C, D)) / np.sqrt(C)).astype(np.float32),
        "Wk": (rng.standard_normal((C, D)) / np.sqrt(C)).astype(np.float32),
        "Wv": (rng.standard_normal((C, D)) / np.sqrt(C)).astype(np.float32),
        "bq": np.zeros(D, np.float32),
        "bk": np.zeros(D, np.float32),
        "bv": np.zeros(D, np.float32),
    }
    out = kernel(**ins)
    print("out shape:", out.shape, "finite:", np.isfinite(out).all())


# revision 8
# speedup vs baseline: 1.3784x; 1.3784x over previous
"""MultiHeadAttention Trainium2 Bass kernel.

Problem: B=8, H=W=32 (S=1024), C=512, 8 heads x 64 dim.
Sharding: data-parallel over batch, one batch element per NeuronCore (8 cores).

Per-core pipeline (batch b):
  Phase A (projections): for x in {q,k,v}: DMA x [1024,512] -> PE-transpose
    to xT [c,s] (fp32, 2cyc/row) -> float32r matmuls with W stationary:
      QT[d,s], KT[d,s] (head dims on partitions), V[s,d] natural (+ones col).
    Biases folded in as K=1 rank-1 matmul accumulated into the same PSUM.
  Phase B (attention, per head pair / q-half): scoresT[k,q] = KT_h^T-stationary
    matmul (K=64, head pairs packed in PE row groups); exp via ACT straight
    from PSUM with the 1/8 scale folded in (no max-subtraction: scores ~N(0,1));
    att@V as V_aug-stationary matmul accumulating over k chunks, the ones
    column producing the softmax denominator for free; PE back-transpose of
    [65, q] tiles; DVE reciprocal + per-partition scale into staged output.
  Phase C: one batched 2MB output DMA.

float32r: single-pass fp32 matmul mode (1 col/cycle at N>=256, vs 4 for
fp32); operands must be written by rounding producers (DVE copies with f32r
output dtype). Measured accuracy ~7e-4 absmax on unit-scale data.
"""
import sys

import numpy as np

if "/opt/trn_rl_repo" not in sys.path:
    sys.path.insert(0, "/opt/trn_rl_repo")

import concourse.bacc as bacc
import concourse.mybir as mybir
import concourse.tile as tile
from concourse import masks
from concourse.bass_utils import run_bass_kernel_spmd

B, HS, WS, C = 8, 32, 32, 512
S = HS * WS          # 1024
D = 512
HEADS = 8
HD = 64              # head dim
N_CORES = 8

f32 = mybir.dt.float32
f32r = mybir.dt.float32r
bf16 = mybir.dt.bfloat16
Exp = mybir.ActivationFunctionType.Exp


def build_nc():
    nc = bacc.Bacc("TRN2", target_bir_lowering=False, debug=False,
                   num_devices=N_CORES)

    x_d = {}
    w_d = {}
    b_d = {}
    for name in ("q", "k", "v"):
        x_d[name] = nc.dram_tensor(f"{name}_in", [S, C], f32, kind="ExternalInput")
        w_d[name] = nc.dram_tensor(f"W{name}", [C, D], f32, kind="ExternalInput")
        b_d[name] = nc.dram_tensor(f"b{name}", [D], f32, kind="ExternalInput")
    out_d = nc.dram_tensor("out", [S, D], f32, kind="ExternalOutput")

    with tile.TileContext(nc) as tc:
        with (
            tc.tile_pool(name="const", bufs=1) as cpool,
            tc.tile_pool(name="xin", bufs=2) as xin_pool,
            tc.tile_pool(name="wbuf", bufs=2) as w_pool,
            tc.tile_pool(name="proj", bufs=1) as proj_pool,
            tc.tile_pool(name="att", bufs=1) as att_pool,
            tc.tile_pool(name="ot", bufs=3) as ot_pool,
            tc.tile_pool(name="ostage", bufs=1) as o_pool,
        ):
            ident = cpool.tile([128, 128], f32)
            masks.make_identity(nc, ident[:])
            ones_sb = cpool.tile([128, 512], f32)
            nc.vector.memset(ones_sb[:], 1.0)
            ones_r = cpool.tile([1, 512], f32r)
            nc.vector.tensor_copy(ones_r[:], ones_sb[0:1, :])

            # Persistent projection outputs
            QT = proj_pool.tile([128, 4, S], bf16, name="QT")   # [d%128, d//128, s]
            KT = proj_pool.tile([128, 4, S], bf16, name="KT")
            # V_aug: [s%128, s//128, head, 65]; col 64 = 1.0 (denominator)
            V = proj_pool.tile([128, 8, HEADS, HD + 1], bf16, name="V")
            nc.vector.tensor_copy(
                V[:, :, :, HD:HD + 1],
                ones_sb[:, 0:64].rearrange("p (a b o) -> p a b o", a=8, b=8))
            o_stage = o_pool.tile([128, 8, D], f32, name="o_stage")

            # ---------------- Phase A: projections ----------------
            with (
                tc.tile_pool(name="ps_tr", bufs=2, space="PSUM") as ps_tr,
                tc.tile_pool(name="ps_proj", bufs=3, space="PSUM") as ps_proj,
                tc.tile_pool(name="xT", bufs=2) as xt_pool,
            ):
                for name, tgt in (("q", QT), ("k", KT), ("v", V)):
                    x_r = x_d[name][:].rearrange("(t p) c -> p t c", p=128)
                    w_sb = w_pool.tile([128, 4, D], f32, name=f"w_{name}",
                                       tag="w_sb")
                    nc.sync.dma_start(
                        w_sb[:], w_d[name][:].rearrange("(cc p) d -> p cc d", p=128))
                    w_r = w_pool.tile([128, 4, D], f32r, name=f"wr_{name}",
                                      tag="w_r")
                    nc.vector.tensor_copy(w_r[:], w_sb[:])
                    b_sb = w_pool.tile([1, D], f32, name=f"b_{name}", tag="b_sb")
                    nc.sync.dma_start(b_sb[:], b_d[name][:].rearrange("(o d) -> o d", o=1))
                    b_r = w_pool.tile([1, D], f32r, name=f"br_{name}", tag="b_r")
                    nc.vector.tensor_copy(b_r[:], b_sb[:])

                    # transpose x -> xT [c%128, cc, s] (f32r via DVE rounding copy)
                    xT = xt_pool.tile([128, 4, S], f32r, name=f"xT_{name}",
                                      tag="xT")
                    for sh in range(2):
                        x_sb = xin_pool.tile([128, 4, C], f32,
                                             name=f"x_{name}{sh}", tag="x_sb")
                        nc.sync.dma_start(x_sb[:], x_r[:, sh * 4:(sh + 1) * 4, :])
                        for ti in range(4):
                            t = sh * 4 + ti
                            pst = ps_tr.tile([128, 4, 128], f32, tag="pst")
                            for cc in range(4):
                                nc.tensor.transpose(
                                    pst[:, cc, :],
                                    x_sb[:, ti, cc * 128:(cc + 1) * 128], ident[:])
                            nc.vector.tensor_copy(
                                xT[:, :, t * 128:(t + 1) * 128], pst[:])

                    if name in ("q", "k"):
                        # tgt[d%128, dt, s] = sum_cc W[cc,dt].T @ xT[cc, s] + b
                        for dt in range(4):
                            for qh in range(2):
                                psq = ps_proj.tile([128, 512], f32, tag="psq")
                                for cc in range(4):
                                    nc.tensor.matmul(
                                        psq[:],
                                        w_r[:, cc, dt * 128:(dt + 1) * 128],
                                        xT[:, cc, qh * 512:(qh + 1) * 512],
                                        start=(cc == 0), stop=False)
                                nc.tensor.matmul(
                                    psq[:],
                                    b_r[0:1, dt * 128:(dt + 1) * 128],
                                    ones_r[0:1, :],
                                    start=False, stop=True)
                                nc.vector.tensor_copy(
                                    tgt[:, dt, qh * 512:(qh + 1) * 512], psq[:])
                    else:
                        # V natural: [s%128, st, d] = xT[cc, st].T-stationary
                        for st in range(8):
                            psv = ps_proj.tile([128, 512], f32, tag="psq")
                            for cc in range(4):
                                nc.tensor.matmul(
                                    psv[:],
                                    xT[:, cc, st * 128:(st + 1) * 128],
                                    w_r[:, cc, :],
                                    start=(cc == 0), stop=False)
                            nc.tensor.matmul(
                                psv[:], ones_r[0:1, 0:128], b_r[0:1, :],
                                start=False, stop=True)
                            nc.vector.tensor_copy(
                                tgt[:, st, :, 0:HD],
                                psv[:].rearrange("p (h e) -> p h e", h=HEADS))

            # ---------------- Phase B: attention ----------------
            with (
                tc.tile_pool(name="ps_s", bufs=1, space="PSUM") as ps_s,
                tc.tile_pool(name="ps_o", bufs=1, space="PSUM") as ps_o,
                tc.tile_pool(name="ps_bt", bufs=2, space="PSUM") as ps_bt,
            ):
                for hp in range(4):          # head pairs share a d-chunk
                    heads = (2 * hp, 2 * hp + 1)
                    for qh in range(2):      # q halves of 512
                        attT = {}
                        for i, h in enumerate(heads):
                            attT[h] = att_pool.tile(
                                [128, 8, 512], bf16, name=f"attT{h}_{qh}",
                                tag=f"attT{i}")
                        # scoresT: per kt pair, two heads packed in row groups
                        for ktp in range(4):
                            pss = {}
                            for i, h in enumerate(heads):
                                pss[h] = ps_s.tile([128, 2, 512], f32,
                                                   name=f"pss{h}_{qh}_{ktp}",
                                                   tag=f"pss{i}")
                            for kt2 in range(2):
                                kt = ktp * 2 + kt2
                                for h in heads:
                                    po = (h % 2) * HD
                                    nc.tensor.matmul(
                                        pss[h][:, kt2, :],
                                        KT[po:po + HD, hp, kt * 128:(kt + 1) * 128],
                                        QT[po:po + HD, hp, qh * 512:(qh + 1) * 512],
                                        start=True, stop=True)
                            for h in heads:
                                nc.scalar.activation(
                                    attT[h][:, ktp * 2:ktp * 2 + 2, :],
                                    pss[h][:], Exp, scale=0.125)
                        # att @ V_aug (+denominator row)
                        pso = {}
                        for i, h in enumerate(heads):
                            pso[h] = ps_o.tile([HD + 1, 512], f32,
                                               name=f"pso{h}_{qh}", tag=f"pso{i}")
                        for kc in range(8):
                            for h in heads:
                                nc.tensor.matmul(
                                    pso[h][:],
                                    V[:, kc, h, :],
                                    attT[h][:, kc, :],
                                    start=(kc == 0), stop=(kc == 7))
                        for h in heads:
                            oT = ot_pool.tile([HD + 1, 512], f32, tag="oT")
                            nc.vector.tensor_copy(oT[:], pso[h][:])
                            # back-transpose 4 q-subtiles into one PSUM bank
                            pbt = ps_bt.tile([128, 4, HD + 1], f32, tag="pbt")
                            for qs in range(4):
                                nc.tensor.transpose(
                                    pbt[:, qs, :],
                                    oT[:, qs * 128:(qs + 1) * 128],
                                    ident[0:HD + 1, 0:HD + 1])
                            rec = ot_pool.tile([128, 4], f32, tag="rec")
                            nc.vector.reciprocal(rec[:], pbt[:, :, HD])
                            for qs in range(4):
                                qt = qh * 4 + qs
                                nc.vector.tensor_scalar_mul(
                                    o_stage[:, qt, h * HD:(h + 1) * HD],
                                    pbt[:, qs, 0:HD],
                                    rec[:, qs:qs + 1])

            # ---------------- Phase C: output ----------------
            nc.sync.dma_start(
                out_d[:].rearrange("(t p) d -> p t d", p=128), o_stage[:])

    nc.compile()
    return nc


_NC = None


def _get_nc():
    global _NC
    if _NC is None:
        _NC = build_nc()
    return _NC


def _make_in_maps(inputs):
    in_maps = []
    for b in range(B):
        m = {
            "q_in": np.ascontiguousarray(inputs["q_in"][b].reshape(S, C)),
            "k_in": np.ascontiguousarray(inputs["k_in"][b].reshape(S, C)),
            "v_in": np.ascontiguousarray(inputs["v_in"][b].reshape(S, C)),
            "Wq": np.asarray(inputs["Wq"]), "bq": np.asarray(inputs["bq"]),
            "Wk": np.asarray(inputs["Wk"]), "bk": np.asarray(inputs["bk"]),
            "Wv": np.asarray(inputs["Wv"]), "bv": np.asarray(inputs["bv"]),
        }
        in_maps.append(m)
    return in_maps


def kernel(**inputs):
    nc = _get_nc()
    res = run_bass_kernel_spmd(nc, _make_in_maps(inputs), list(range(N_CORES)))
    out = np.stack([res.results[i]["out"] for i in range(B)])
    return out.reshape(B, HS, WS, D).astype(np.float32)


if __name__ == "__main__":
    rng = np.random.default_rng(0)
    ins = {
        "q_in": rng.standard_normal((B, HS, WS, C), dtype=np.float32),
        "k_in": rng.standard_normal((B, HS, WS, C), dtype=np.float32),
        "v_in": rng.standard_normal((B, HS, WS, C), dtype=np.float32),
        "Wq": (rng.standard_normal((C, D)) / np.sqrt(C)).astype(np.float32),
        "Wk": (rng.standard_normal((C, D)) / np.sqrt(C)).astype(np.float32),
        "Wv": (rng.standard_normal((C, D)) / np.sqrt(C)).astype(np.float32),
        "bq": np.zeros(D, np.float32),
        "bk": np.zeros(D, np.float32),
        "bv": np.zeros(D, np.float32),
    }
    out = kernel(**ins)
    print("out shape:", out.shape, "finite:", np.isfinite(out).all())
